# revision 40
# baseline (speedup 1.0000x reference)
"""Trainium2 Bass kernel for BaseAttentionEntityPooler.

Problem (F=1, B=16, L=4096, D=2048, T=4 spans per batch, span length < 16):
  mask  = union of T index spans per (f, b)                      [F,B,L]
  score = concat(ent*m, hidden*m) @ W_align.T + b_align          [B,L,1]
  probs = softmax(score masked to spans)                         [B,L]
  pooled = sum_l probs * hidden                                  [B,D]
  projected = tanh(pooled @ W_out.T + b_out)                     [B,D]
  returns (projected, attn=probs[...,None])

Only <=64 positions per batch carry attention mass, so the kernel never
streams the 512 MiB `hidden`: each core gathers the T*16-row span windows of
its 2 batches with an indirect DMA (1 MiB), computes scores/softmax on the
128 gathered rows, and scatters the probabilities back into the (pre-zeroed)
attention output. A second tiny SPMD kernel computes the output projection
with W_out sharded over cores (256 rows each).

Sharding: data-parallel over batch B for the pooling kernel (hidden,
token_idxs, pooled_entities split across 8 cores; alignment weights
replicated); output-dim-parallel over D_out for the projection kernel
(pooled replicated, W_out/b_out split).
"""

import os
import sys

for _p in (
    "/root/.axon_site",
    "/root/.axon_site/_ro/trn_rl_repo",
    "/root/.axon_site/_ro/pypackages",
    "/opt/trn_rl_repo",
):
    if os.path.isdir(_p) and _p not in sys.path:
        sys.path.append(_p)

import numpy as np

import concourse.bass as bass
import concourse.tile as tile
from concourse import mybir

F, B, L, D, T, W = 1, 16, 4096, 2048, 4, 16
NCORES = 8
BL = B // NCORES        # local batches per core
S = BL * T * W          # gathered slots per core (= 128 partitions)
DC = D // 128           # 128-column chunks of D
OSH = D // NCORES       # W_out rows per core
TW = T * W              # slots per batch

f32 = mybir.dt.float32
bf16 = mybir.dt.bfloat16
i32 = mybir.dt.int32

NEG = -1.0e30


def _legalize_multi_waits(nc):
    """The TPB ISA gives every instruction exactly one semaphore wait slot and
    this walrus build refuses instructions carrying more. Hoist all but the
    last wait of each instruction into wait-only EventSemaphore instructions
    placed immediately before it on the same engine (the sequencer stalls on
    those first, which is semantically identical)."""
    n = 0
    for fn in nc.m.functions:
        for bb in fn.blocks:
            new_insts = []
            changed = False
            for ins in bb.instructions:
                si = ins.sync_info
                if si is not None and si.on_wait and len(si.on_wait) > 1:
                    waits = list(si.on_wait)
                    for w in waits[:-1]:
                        ev = mybir.InstEventSemaphore(
                            name=f"hoistw_{n}", ins=[], outs=[]
                        )
                        n += 1
                        ev.engine = ins.engine
                        ev.sync_info = mybir.SyncInfo(on_wait=[w], on_update=[])
                        nc.register_instruction(ev, overwrite=True)
                        new_insts.append(ev)
                    ins.sync_info = mybir.SyncInfo(
                        on_wait=[waits[-1]], on_update=list(si.on_update or [])
                    )
                    changed = True
                if si is not None and si.on_update and len(si.on_update) > 1:
                    raise AssertionError(f"{ins.name}: >1 sync updates not handled")
                new_insts.append(ins)
            if changed:
                bb.instructions = new_insts
    return nc


def _build_pool_kernel(debug=False):
    """Per-core sparse attention pooling.

    Inputs:  hid [BL*L, D], tok [BL, T, 2] i32, jc [S, 1] i32 (host constant:
             slot p -> p%W + (p//TW)*L), ent [BL, D], wal [1, 2D], bal [1, 1]
    Outputs: pooled [BL, D], attn [BL*L, 1] (relies on pre-zeroed outputs)
    """
    nc = bass.Bass()
    dbg = {}
    if debug:
        for nm, shp, dt in [
            ("d_idxp", [S, 1], i32), ("d_s2", [BL, TW], f32),
            ("d_un", [BL, TW], f32), ("d_valid", [BL, TW], f32),
            ("d_pv", [BL, TW], f32), ("d_wsc", [BL, TW], f32),
            ("d_cb", [BL, 1], f32), ("d_mx", [BL, 1], f32),
            ("d_zs", [BL, 1], f32), ("d_probs2", [S, BL], f32),
            ("d_rows", [S, 8], f32),
        ]:
            dbg[nm] = nc.dram_tensor(nm, shp, dt, kind="ExternalOutput")

    hid = nc.dram_tensor("hid", [BL * L, D], f32, kind="ExternalInput")
    tok = nc.dram_tensor("tok", [BL, T, 2], i32, kind="ExternalInput")
    jc = nc.dram_tensor("jc", [S, 1], i32, kind="ExternalInput")
    ent = nc.dram_tensor("ent", [BL, D], f32, kind="ExternalInput")
    wal = nc.dram_tensor("wal", [1, 2 * D], f32, kind="ExternalInput")
    bal = nc.dram_tensor("bal", [1, 1], f32, kind="ExternalInput")
    pooledT = nc.dram_tensor("pooledT", [128, DC * BL], bf16, kind="ExternalOutput")
    attn = nc.dram_tensor("attn", [BL * L, 1], f32, kind="ExternalOutput")

    with tile.TileContext(nc) as tc:
        with (
            tc.tile_pool(name="big", bufs=1) as big,
            tc.tile_pool(name="small", bufs=1) as small,
            tc.tile_pool(name="ps", bufs=1, space="PSUM") as ps,
        ):
            # ---- gather indices straight in partition layout ----
            startp = small.tile([S, 1], i32)
            nc.sync.dma_start(
                out=startp[:],
                in_=bass.AP(
                    tensor=tok[:].tensor, offset=0,
                    ap=[[2 * T, BL], [2, T], [0, W], [1, 1]],
                ),
            )
            jcp = small.tile([S, 1], i32)
            nc.sync.dma_start(out=jcp[:], in_=jc[:])
            idxp = small.tile([S, 1], i32)
            nc.gpsimd.tensor_add(idxp[:], startp[:], jcp[:])

            # ---- gather span windows of hidden (1 MiB) ----
            # column-split into two tiles so the score matvec can start on
            # the first half while the second half is still in flight
            HD = D // 2
            rows0 = big.tile([S, HD], f32, tag="rows0")
            rows1 = big.tile([S, HD], f32, tag="rows1")
            nc.gpsimd.indirect_dma_start(
                out=rows0[:],
                out_offset=None,
                in_=hid[:],
                in_offset=bass.IndirectOffsetOnAxis(ap=idxp[:, 0:1], axis=0),
            )
            nc.gpsimd.indirect_dma_start(
                out=rows1[:],
                out_offset=None,
                in_=hid[:],
                in_offset=bass.IndirectOffsetOnAxis(ap=idxp[:, 0:1], axis=0),
                element_offset=HD,
            )

            # broadcast of W_align[D:] across partitions, overlaps the gather
            wvec = big.tile([S, D], f32)
            nc.sync.dma_start(out=wvec[:], in_=wal[:, D:].to_broadcast([S, D]))

            # ---- per-batch bias: ent . W_align[:D] (wide layout) ----
            # small DMAs go out on the scalar HWDGE ring so they don't
            # serialize behind wvec on the sync ring
            ent32 = small.tile([BL * 16, 128], f32)
            nc.scalar.dma_start(
                out=ent32[:], in_=ent[:].rearrange("b (c p) -> (b c) p", p=128)
            )
            went = small.tile([BL * 16, 128], f32)
            nc.scalar.dma_start(
                out=went[:],
                in_=bass.AP(
                    tensor=wal[:].tensor, offset=0,
                    ap=[[0, BL], [128, 16], [1, 128]],
                ),
            )
            entp = small.tile([BL * 16, 128], f32)
            entr = small.tile([BL * 16, 1], f32)
            nc.vector.tensor_mul(entp[:], ent32[:], went[:])
            nc.vector.reduce_sum(entr[:], entp[:], axis=mybir.AxisListType.X)
            cb16 = small.tile([BL, 16], f32)
            nc.scalar.dma_start(out=cb16[:], in_=entr[:])
            cb = small.tile([BL, 1], f32)
            nc.vector.reduce_sum(cb[:], cb16[:], axis=mybir.AxisListType.X)

            balb = small.tile([BL, 1], f32)
            nc.scalar.dma_start(out=balb[:], in_=bal[:, 0:1].to_broadcast([BL, 1]))

            # ---- slot positions and masks in [BL, TW] layout ----
            # high priority: this chain should fill DVE while the gather is
            # in flight, not trail the score matvec
            with tc.high_priority():
                t_tok = small.tile([BL, T, 2], i32)
                nc.scalar.dma_start(out=t_tok[:], in_=tok[:])
                jj = small.tile([BL, TW], i32)
                nc.gpsimd.iota(
                    jj[:].rearrange("b (t w) -> b t w", t=T),
                    pattern=[[0, T], [1, W]], base=0, channel_multiplier=0,
                )
                startb = small.tile([BL, T, W], i32)
                nc.vector.tensor_copy(
                    startb[:], t_tok[:, :, 0:1].to_broadcast([BL, T, W])
                )
                pos64 = small.tile([BL, TW], i32)
                nc.vector.tensor_add(
                    pos64[:], startb[:].rearrange("b t w -> b (t w)"), jj[:]
                )
                endown = small.tile([BL, T, W], i32)
                nc.vector.tensor_copy(
                    endown[:], t_tok[:, :, 1:2].to_broadcast([BL, T, W])
                )

                # per-slot x per-span membership [BL, TW, T]
                pos3 = bass.AP(
                    tensor=pos64[:].tensor,
                    offset=pos64[:].offset,
                    ap=list(pos64[:].ap) + [[0, T]],
                )
                st3 = (
                    t_tok[:, :, 0:1].rearrange("b t one -> b one t")
                    .to_broadcast([BL, TW, T])
                )
                en3 = (
                    t_tok[:, :, 1:2].rearrange("b t one -> b one t")
                    .to_broadcast([BL, TW, T])
                )
                ge4 = small.tile([BL, TW, T], f32)
                lt4 = small.tile([BL, TW, T], f32)
                in4 = small.tile([BL, TW, T], f32)
                nc.vector.tensor_tensor(
                    out=ge4[:], in0=pos3, in1=st3, op=mybir.AluOpType.is_ge
                )
                nc.vector.tensor_tensor(
                    out=lt4[:], in0=pos3, in1=en3, op=mybir.AluOpType.is_lt
                )
                nc.vector.tensor_mul(in4[:], ge4[:], lt4[:])
                un = small.tile([BL, TW], f32)
                nc.vector.reduce_max(un[:], in4[:], axis=mybir.AxisListType.X)

                valid = small.tile([BL, TW], f32)
                nc.vector.tensor_tensor(
                    out=valid[:], in0=pos64[:],
                    in1=endown[:].rearrange("b t w -> b (t w)"),
                    op=mybir.AluOpType.is_lt,
                )
                notin4 = small.tile([BL, TW, T], f32)
                nc.vector.tensor_scalar(
                    out=notin4[:], in0=in4[:], scalar1=-1.0, scalar2=1.0,
                    op0=mybir.AluOpType.mult, op1=mybir.AluOpType.add,
                )
                for s in range(T - 1):
                    lo = (s + 1) * W
                    nv = notin4[:, lo:, s : s + 1].rearrange("b k one -> b (k one)")
                    nc.vector.tensor_mul(valid[:, lo:], valid[:, lo:], nv)

            # ---- slot scores: rows . W_align[D:], pipelined per half ----
            prod = big.tile([S, D], f32)
            scpa = small.tile([S, 1], f32)
            scpb = small.tile([S, 1], f32)
            scp = small.tile([S, 1], f32)
            nc.vector.tensor_mul(prod[:, 0:HD], rows0[:], wvec[:, 0:HD])
            nc.vector.reduce_sum(scpa[:], prod[:, 0:HD], axis=mybir.AxisListType.X)
            nc.vector.tensor_mul(prod[:, HD:], rows1[:], wvec[:, HD:])
            nc.vector.reduce_sum(scpb[:], prod[:, HD:], axis=mybir.AxisListType.X)
            nc.vector.tensor_add(scp[:], scpa[:], scpb[:])
            s2 = small.tile([BL, TW], f32)
            nc.sync.dma_start(out=s2[:], in_=scp[:])

            # ---- masked softmax over slots ----
            tmpa = small.tile([BL, TW], f32)
            sin = small.tile([BL, TW], f32)
            nc.vector.tensor_scalar(
                out=tmpa[:], in0=valid[:], scalar1=-1.0, scalar2=-NEG,
                op0=mybir.AluOpType.add, op1=mybir.AluOpType.mult,
            )
            nc.vector.tensor_add(sin[:], s2[:], tmpa[:])
            mx = small.tile([BL, 1], f32)
            nc.vector.reduce_max(mx[:], sin[:], axis=mybir.AxisListType.X)
            bias = small.tile([BL, 1], f32)
            nc.vector.tensor_sub(bias[:], cb[:], mx[:])
            nc.vector.tensor_add(bias[:], bias[:], balb[:])

            ev = small.tile([BL, TW], f32)
            zsum = small.tile([BL, 1], f32)
            nc.scalar.activation(
                out=ev[:], in_=sin[:], func=mybir.ActivationFunctionType.Exp,
                bias=bias[:, 0:1], scale=1.0, accum_out=zsum[:],
            )
            ea = small.tile([BL, TW], f32)
            nc.scalar.activation(
                out=ea[:], in_=s2[:], func=mybir.ActivationFunctionType.Exp,
                bias=bias[:, 0:1], scale=1.0,
            )
            rz = small.tile([BL, 1], f32)
            nc.vector.reciprocal(rz[:], zsum[:])

            pv = small.tile([BL, TW], bf16)
            nc.vector.tensor_scalar(
                out=pv[:], in0=ev[:], scalar1=rz[:, 0:1], scalar2=None,
                op0=mybir.AluOpType.mult,
            )
            wsc = small.tile([BL, TW], f32)
            nc.vector.tensor_scalar(
                out=wsc[:], in0=ea[:], scalar1=rz[:, 0:1], scalar2=None,
                op0=mybir.AluOpType.mult,
            )
            nc.vector.tensor_mul(wsc[:], wsc[:], un[:])

            # ---- pooled[b, :] = probs_b @ rows_b  (probs as tiny lhsT) ----
            # matmul runs in bf16 (fp32 PE streaming is ~8x slower); the
            # rows cast happens on the scalar engine while DVE does scores
            rows_bf = big.tile([S, D], bf16)
            nc.scalar.copy(rows_bf[:, 0:HD], rows0[:])
            nc.scalar.copy(rows_bf[:, HD:], rows1[:])
            probs2 = small.tile([S, BL], bf16)
            nc.gpsimd.memset(probs2[:], 0.0)
            for b in range(BL):
                nc.sync.dma_start(
                    out=probs2[b * TW : (b + 1) * TW, b : b + 1],
                    in_=pv[b : b + 1, :],
                )
            # pooled in d-major layout: rows as stationary per-chunk weights
            pl = ps.tile([128, DC * BL], f32)
            for c in range(DC):
                nc.tensor.matmul(
                    pl[:, c * BL : (c + 1) * BL],
                    lhsT=rows_bf[:, c * 128 : (c + 1) * 128],
                    rhs=probs2[:],
                    start=True,
                    stop=True,
                )
            plsb = small.tile([128, DC * BL], bf16)
            nc.vector.tensor_copy(plsb[:], pl[:])
            nc.sync.dma_start(out=pooledT[:], in_=plsb[:])

            # ---- scatter attention probabilities ----
            wp = small.tile([S, 1], f32)
            nc.sync.dma_start(out=wp[:], in_=wsc[:])
            nc.gpsimd.indirect_dma_start(
                out=attn[:],
                out_offset=bass.IndirectOffsetOnAxis(ap=idxp[:, 0:1], axis=0),
                in_=wp[:],
                in_offset=None,
            )

            if debug:
                for nm, src in [
                    ("d_idxp", idxp[:]), ("d_s2", s2[:]), ("d_un", un[:]),
                    ("d_valid", valid[:]), ("d_pv", pv[:]), ("d_wsc", wsc[:]),
                    ("d_cb", cb[:]), ("d_mx", mx[:]), ("d_zs", zsum[:]),
                    ("d_probs2", probs2[:]), ("d_rows", rows0[:, 0:8]),
                ]:
                    nc.sync.dma_start(out=dbg[nm][:], in_=src)

    return _legalize_multi_waits(nc)


def _build_proj_kernel():
    """Per-core slice of the output projection.

    Inputs:  pooledb [128, DC*B]   (pooledb[p, c*B+b] = pooled[b, c*128+p])
             wob [128, DC*OSH]     (wob[p, c*OSH+o] = W_out[o_base+o, c*128+p])
             bob [1, OSH]
    Output:  projT [B, OSH]        (projT[b, o] = projected[b, o_base+o])
    """
    nc = bass.Bass()
    pooledb = nc.dram_tensor("pooledb", [128, DC * B], bf16, kind="ExternalInput")
    wob = nc.dram_tensor("wob", [128, DC * OSH], bf16, kind="ExternalInput")
    bob = nc.dram_tensor("bob", [1, OSH], f32, kind="ExternalInput")
    projT = nc.dram_tensor("projT", [B, OSH], f32, kind="ExternalOutput")

    NQ = 4
    CQ = DC // NQ
    with tile.TileContext(nc) as tc:
        with (
            tc.tile_pool(name="pool", bufs=1) as pool,
            tc.tile_pool(name="ps", bufs=1, space="PSUM") as ps,
        ):
            pb = pool.tile([128, DC, B], bf16)
            nc.sync.dma_start(
                out=pb[:], in_=pooledb[:].rearrange("p (c b) -> p c b", b=B)
            )
            wq = []
            for q in range(NQ):
                t = pool.tile([128, CQ, OSH], bf16, tag=f"w{q}")
                eng = nc.sync if q % 2 == 0 else nc.scalar
                eng.dma_start(
                    out=t[:],
                    in_=wob[:, q * CQ * OSH : (q + 1) * CQ * OSH].rearrange(
                        "p (c o) -> p c o", o=OSH
                    ),
                )
                wq.append(t)
            # bias folded into the PSUM group as a K=1 ones-matmul
            ones1 = pool.tile([1, B], bf16)
            nc.vector.memset(ones1[:], 1.0)
            bobs = pool.tile([1, OSH], bf16)
            nc.gpsimd.dma_start(out=bobs[:], in_=bob[0:1, :])

            pj = ps.tile([B, OSH], f32)
            nc.tensor.matmul(pj[:], lhsT=ones1[:], rhs=bobs[:], start=True, stop=False)
            for c in range(DC):
                q, r = divmod(c, CQ)
                nc.tensor.matmul(
                    pj[:],
                    lhsT=pb[:, c, :],
                    rhs=wq[q][:, r, :],
                    start=False,
                    stop=(c == DC - 1),
                )
            ot2 = pool.tile([B, OSH], f32)
            nc.scalar.activation(
                out=ot2[:], in_=pj[:], func=mybir.ActivationFunctionType.Tanh
            )
            nc.sync.dma_start(out=projT[:], in_=ot2[:])

    return _legalize_multi_waits(nc)


_POOL_NC = None
_PROJ_NC = None


def _get_kernels():
    global _POOL_NC, _PROJ_NC
    if _POOL_NC is None:
        _POOL_NC = _build_pool_kernel()
        _PROJ_NC = _build_proj_kernel()
    return _POOL_NC, _PROJ_NC


def _jc_const():
    p = np.arange(S, dtype=np.int32)
    return ((p % W) + (p // TW) * L).reshape(S, 1)


def _pool_in_maps(hidden, token_idxs, pooled_entities, W_align, b_align):
    wal = np.ascontiguousarray(W_align, dtype=np.float32).reshape(1, 2 * D)
    bal = np.ascontiguousarray(b_align, dtype=np.float32).reshape(1, 1)
    tok = np.ascontiguousarray(token_idxs.reshape(B, T, 2), dtype=np.int32)
    jc = _jc_const()
    maps = []
    for g in range(NCORES):
        b0 = g * BL
        maps.append(
            {
                "hid": np.ascontiguousarray(
                    hidden[b0 : b0 + BL].reshape(BL * L, D), dtype=np.float32
                ),
                "tok": tok[b0 : b0 + BL],
                "jc": jc,
                "ent": np.ascontiguousarray(
                    pooled_entities[b0 : b0 + BL], dtype=np.float32
                ),
                "wal": wal,
                "bal": bal,
            }
        )
    return maps


def _proj_in_maps(pooled_all, W_out, b_out):
    import ml_dtypes

    # pooledb[p, c*B+b] = pooled_all[b, c*128+p]
    pooledb = np.ascontiguousarray(
        pooled_all.T.reshape(DC, 128, B).transpose(1, 0, 2).reshape(128, DC * B)
    ).astype(ml_dtypes.bfloat16)
    maps = []
    for g in range(NCORES):
        o0 = g * OSH
        wob = np.ascontiguousarray(
            W_out[o0 : o0 + OSH]
            .T.reshape(DC, 128, OSH)
            .transpose(1, 0, 2)
            .reshape(128, DC * OSH)
        ).astype(ml_dtypes.bfloat16)
        maps.append(
            {
                "pooledb": pooledb,
                "wob": wob,
                "bob": np.ascontiguousarray(
                    b_out[o0 : o0 + OSH], dtype=np.float32
                ).reshape(1, OSH),
            }
        )
    return maps


LAST_EXEC_NS = []


def kernel(hidden, token_idxs, pooled_entities, W_align, b_align, W_out, b_out):
    from concourse.bass_utils import run_bass_kernel_spmd

    LAST_EXEC_NS.clear()
    hidden = np.asarray(hidden, dtype=np.float32)
    token_idxs = np.asarray(token_idxs)
    pooled_entities = np.asarray(pooled_entities, dtype=np.float32)
    W_align = np.asarray(W_align, dtype=np.float32)
    b_align = np.asarray(b_align, dtype=np.float32)
    W_out = np.asarray(W_out, dtype=np.float32)
    b_out = np.asarray(b_out, dtype=np.float32)

    if int(np.max(token_idxs[..., 1] - token_idxs[..., 0])) > W:
        # Out-of-contract input (setup_inputs caps span length at 16);
        # fall back to a plain numpy evaluation rather than return garbage.
        return _numpy_reference(
            hidden, token_idxs, pooled_entities, W_align, b_align, W_out, b_out
        )

    pool_nc, proj_nc = _get_kernels()
    core_ids = list(range(NCORES))

    kr_a = run_bass_kernel_spmd(
        pool_nc,
        _pool_in_maps(hidden, token_idxs, pooled_entities, W_align, b_align),
        core_ids,
    )
    res_a = kr_a.results
    LAST_EXEC_NS.append(kr_a.exec_time_ns)

    pooled_all = np.empty((B, D), dtype=np.float32)
    for g in range(NCORES):
        pt = np.asarray(res_a[g]["pooledT"], dtype=np.float32).reshape(128, DC, BL)
        for b in range(BL):
            pooled_all[g * BL + b] = pt[:, :, b].T.reshape(D)
    attn = np.empty((F, B, L, 1), dtype=np.float32)
    for g in range(NCORES):
        attn[0, g * BL : (g + 1) * BL, :, 0] = res_a[g]["attn"].reshape(BL, L)

    kr_b = run_bass_kernel_spmd(
        proj_nc, _proj_in_maps(pooled_all, W_out, b_out), core_ids
    )
    res_b = kr_b.results
    LAST_EXEC_NS.append(kr_b.exec_time_ns)

    projected = np.empty((B, D), dtype=np.float32)
    for g in range(NCORES):
        projected[:, g * OSH : (g + 1) * OSH] = res_b[g]["projT"]
    return projected, attn


def _numpy_reference(hidden, token_idxs, pooled_entities, W_align, b_align, W_out, b_out):
    pos = np.arange(L)
    starts = token_idxs[..., 0][..., None]
    ends = token_idxs[..., 1][..., None]
    mask = ((pos >= starts) & (pos < ends)).any(axis=2)  # [F,B,L]
    attn = np.zeros((F, B, L, 1), dtype=np.float32)
    pooled_f = []
    for f in range(F):
        m = mask[f][..., None].astype(np.float32)
        mh = hidden * m
        er = pooled_entities[:, None, :] * m
        scores = np.concatenate([er, mh], axis=-1) @ W_align.T + b_align
        s = np.where(mask[f], scores[..., 0], -np.inf)
        s = s - s.max(axis=-1, keepdims=True)
        e = np.exp(s)
        probs = e / e.sum(axis=-1, keepdims=True)
        probs = np.where(mask[f], probs, 0.0)
        attn[f, :, :, 0] = probs
        pooled_f.append((mh * probs[..., None]).sum(axis=1))
    all_pooled = np.concatenate(pooled_f, axis=1)
    projected = np.tanh(all_pooled @ W_out.T + b_out).astype(np.float32)
    return projected, attn


# revision 41
# speedup vs baseline: 1.0671x; 1.0671x over previous
"""Trainium2 Bass kernel for BaseAttentionEntityPooler.

Problem (F=1, B=16, L=4096, D=2048, T=4 spans per batch, span length < 16):
  mask  = union of T index spans per (f, b)                      [F,B,L]
  score = concat(ent*m, hidden*m) @ W_align.T + b_align          [B,L,1]
  probs = softmax(score masked to spans)                         [B,L]
  pooled = sum_l probs * hidden                                  [B,D]
  projected = tanh(pooled @ W_out.T + b_out)                     [B,D]
  returns (projected, attn=probs[...,None])

Only <=64 positions per batch carry attention mass, so the kernel never
streams the 512 MiB `hidden`: each core gathers the T*16-row span windows of
its 2 batches with an indirect DMA (1 MiB), computes scores/softmax on the
128 gathered rows, and scatters the probabilities back into the (pre-zeroed)
attention output. A second tiny SPMD kernel computes the output projection
with W_out sharded over cores (256 rows each).

Sharding: data-parallel over batch B for the pooling kernel (hidden,
token_idxs, pooled_entities split across 8 cores; alignment weights
replicated); output-dim-parallel over D_out for the projection kernel
(pooled replicated, W_out/b_out split).
"""

import os
import sys

for _p in (
    "/root/.axon_site",
    "/root/.axon_site/_ro/trn_rl_repo",
    "/root/.axon_site/_ro/pypackages",
    "/opt/trn_rl_repo",
):
    if os.path.isdir(_p) and _p not in sys.path:
        sys.path.append(_p)

import numpy as np

import concourse.bass as bass
import concourse.tile as tile
from concourse import mybir

F, B, L, D, T, W = 1, 16, 4096, 2048, 4, 16
NCORES = 8
BL = B // NCORES        # local batches per core
S = BL * T * W          # gathered slots per core (= 128 partitions)
DC = D // 128           # 128-column chunks of D
OSH = D // NCORES       # W_out rows per core
TW = T * W              # slots per batch

f32 = mybir.dt.float32
bf16 = mybir.dt.bfloat16
i32 = mybir.dt.int32

NEG = -1.0e30


def _legalize_multi_waits(nc):
    """The TPB ISA gives every instruction exactly one semaphore wait slot and
    this walrus build refuses instructions carrying more. Hoist all but the
    last wait of each instruction into wait-only EventSemaphore instructions
    placed immediately before it on the same engine (the sequencer stalls on
    those first, which is semantically identical)."""
    n = 0
    for fn in nc.m.functions:
        for bb in fn.blocks:
            new_insts = []
            changed = False
            for ins in bb.instructions:
                si = ins.sync_info
                if si is not None and si.on_wait and len(si.on_wait) > 1:
                    waits = list(si.on_wait)
                    for w in waits[:-1]:
                        ev = mybir.InstEventSemaphore(
                            name=f"hoistw_{n}", ins=[], outs=[]
                        )
                        n += 1
                        ev.engine = ins.engine
                        ev.sync_info = mybir.SyncInfo(on_wait=[w], on_update=[])
                        nc.register_instruction(ev, overwrite=True)
                        new_insts.append(ev)
                    ins.sync_info = mybir.SyncInfo(
                        on_wait=[waits[-1]], on_update=list(si.on_update or [])
                    )
                    changed = True
                if si is not None and si.on_update and len(si.on_update) > 1:
                    raise AssertionError(f"{ins.name}: >1 sync updates not handled")
                new_insts.append(ins)
            if changed:
                bb.instructions = new_insts
    return nc


def _build_pool_kernel(debug=False):
    """Per-core sparse attention pooling.

    Inputs:  hid [BL*L, D], tok [BL, T, 2] i32, jc [S, 1] i32 (host constant:
             slot p -> p%W + (p//TW)*L), ent [BL, D], wal [1, 2D], bal [1, 1]
    Outputs: pooled [BL, D], attn [BL*L, 1] (relies on pre-zeroed outputs)
    """
    nc = bass.Bass()
    dbg = {}
    if debug:
        for nm, shp, dt in [
            ("d_idxp", [S, 1], i32), ("d_s2", [BL, TW], f32),
            ("d_un", [BL, TW], f32), ("d_valid", [BL, TW], f32),
            ("d_pv", [BL, TW], f32), ("d_wsc", [BL, TW], f32),
            ("d_cb", [BL, 1], f32), ("d_mx", [BL, 1], f32),
            ("d_zs", [BL, 1], f32), ("d_probs2", [S, BL], f32),
            ("d_rows", [S, 8], f32),
        ]:
            dbg[nm] = nc.dram_tensor(nm, shp, dt, kind="ExternalOutput")

    hid = nc.dram_tensor("hid", [BL * L, D], f32, kind="ExternalInput")
    tok = nc.dram_tensor("tok", [BL, T, 2], i32, kind="ExternalInput")
    jc = nc.dram_tensor("jc", [S, 1], i32, kind="ExternalInput")
    ent = nc.dram_tensor("ent", [BL, D], f32, kind="ExternalInput")
    wal = nc.dram_tensor("wal", [1, 2 * D], f32, kind="ExternalInput")
    bal = nc.dram_tensor("bal", [1, 1], f32, kind="ExternalInput")
    pooledT = nc.dram_tensor("pooledT", [128, DC * BL], bf16, kind="ExternalOutput")
    attn = nc.dram_tensor("attn", [BL * L, 1], f32, kind="ExternalOutput")

    with tile.TileContext(nc) as tc:
        with (
            tc.tile_pool(name="big", bufs=1) as big,
            tc.tile_pool(name="small", bufs=1) as small,
            tc.tile_pool(name="ps", bufs=1, space="PSUM") as ps,
        ):
            # ---- gather indices straight in partition layout ----
            startp = small.tile([S, 1], i32)
            nc.sync.dma_start(
                out=startp[:],
                in_=bass.AP(
                    tensor=tok[:].tensor, offset=0,
                    ap=[[2 * T, BL], [2, T], [0, W], [1, 1]],
                ),
            )
            jcp = small.tile([S, 1], i32)
            nc.sync.dma_start(out=jcp[:], in_=jc[:])
            idxp = small.tile([S, 1], i32)
            nc.gpsimd.tensor_add(idxp[:], startp[:], jcp[:])

            # ---- gather span windows of hidden (1 MiB) ----
            # column-split into two tiles so the score matvec can start on
            # the first half while the second half is still in flight
            HD = D // 2
            rows0 = big.tile([S, HD], f32, tag="rows0")
            rows1 = big.tile([S, HD], f32, tag="rows1")
            nc.gpsimd.indirect_dma_start(
                out=rows0[:],
                out_offset=None,
                in_=hid[:],
                in_offset=bass.IndirectOffsetOnAxis(ap=idxp[:, 0:1], axis=0),
            )
            nc.gpsimd.indirect_dma_start(
                out=rows1[:],
                out_offset=None,
                in_=hid[:],
                in_offset=bass.IndirectOffsetOnAxis(ap=idxp[:, 0:1], axis=0),
                element_offset=HD,
            )

            # broadcast of W_align[D:] across partitions, overlaps the gather
            wvec = big.tile([S, D], f32)
            nc.sync.dma_start(out=wvec[:], in_=wal[:, D:].to_broadcast([S, D]))

            # ---- per-batch bias: ent . W_align[:D] (wide layout) ----
            # small DMAs go out on the scalar HWDGE ring so they don't
            # serialize behind wvec on the sync ring
            ent32 = small.tile([BL * 16, 128], f32)
            nc.scalar.dma_start(
                out=ent32[:], in_=ent[:].rearrange("b (c p) -> (b c) p", p=128)
            )
            went = small.tile([BL * 16, 128], f32)
            nc.scalar.dma_start(
                out=went[:],
                in_=bass.AP(
                    tensor=wal[:].tensor, offset=0,
                    ap=[[0, BL], [128, 16], [1, 128]],
                ),
            )
            entp = small.tile([BL * 16, 128], f32)
            entr = small.tile([BL * 16, 1], f32)
            nc.vector.tensor_mul(entp[:], ent32[:], went[:])
            nc.vector.reduce_sum(entr[:], entp[:], axis=mybir.AxisListType.X)
            cb16 = small.tile([BL, 16], f32)
            nc.scalar.dma_start(out=cb16[:], in_=entr[:])
            cb = small.tile([BL, 1], f32)
            nc.vector.reduce_sum(cb[:], cb16[:], axis=mybir.AxisListType.X)

            balb = small.tile([BL, 1], f32)
            nc.scalar.dma_start(out=balb[:], in_=bal[:, 0:1].to_broadcast([BL, 1]))

            # ---- slot positions and masks in [BL, TW] layout ----
            # high priority: this chain should fill DVE while the gather is
            # in flight, not trail the score matvec
            with tc.high_priority():
                t_tok = small.tile([BL, T, 2], i32)
                nc.scalar.dma_start(out=t_tok[:], in_=tok[:])
                jj = small.tile([BL, TW], i32)
                nc.gpsimd.iota(
                    jj[:].rearrange("b (t w) -> b t w", t=T),
                    pattern=[[0, T], [1, W]], base=0, channel_multiplier=0,
                )
                startb = small.tile([BL, T, W], i32)
                nc.vector.tensor_copy(
                    startb[:], t_tok[:, :, 0:1].to_broadcast([BL, T, W])
                )
                pos64 = small.tile([BL, TW], i32)
                nc.vector.tensor_add(
                    pos64[:], startb[:].rearrange("b t w -> b (t w)"), jj[:]
                )
                endown = small.tile([BL, T, W], i32)
                nc.vector.tensor_copy(
                    endown[:], t_tok[:, :, 1:2].to_broadcast([BL, T, W])
                )

                # per-slot x per-span membership [BL, TW, T]
                pos3 = bass.AP(
                    tensor=pos64[:].tensor,
                    offset=pos64[:].offset,
                    ap=list(pos64[:].ap) + [[0, T]],
                )
                st3 = (
                    t_tok[:, :, 0:1].rearrange("b t one -> b one t")
                    .to_broadcast([BL, TW, T])
                )
                en3 = (
                    t_tok[:, :, 1:2].rearrange("b t one -> b one t")
                    .to_broadcast([BL, TW, T])
                )
                ge4 = small.tile([BL, TW, T], f32)
                lt4 = small.tile([BL, TW, T], f32)
                in4 = small.tile([BL, TW, T], f32)
                nc.vector.tensor_tensor(
                    out=ge4[:], in0=pos3, in1=st3, op=mybir.AluOpType.is_ge
                )
                nc.vector.tensor_tensor(
                    out=lt4[:], in0=pos3, in1=en3, op=mybir.AluOpType.is_lt
                )
                nc.vector.tensor_mul(in4[:], ge4[:], lt4[:])
                un = small.tile([BL, TW], f32)
                nc.vector.reduce_max(un[:], in4[:], axis=mybir.AxisListType.X)

                valid = small.tile([BL, TW], f32)
                nc.vector.tensor_tensor(
                    out=valid[:], in0=pos64[:],
                    in1=endown[:].rearrange("b t w -> b (t w)"),
                    op=mybir.AluOpType.is_lt,
                )
                notin4 = small.tile([BL, TW, T], f32)
                nc.vector.tensor_scalar(
                    out=notin4[:], in0=in4[:], scalar1=-1.0, scalar2=1.0,
                    op0=mybir.AluOpType.mult, op1=mybir.AluOpType.add,
                )
                for s in range(T - 1):
                    lo = (s + 1) * W
                    nv = notin4[:, lo:, s : s + 1].rearrange("b k one -> b (k one)")
                    nc.vector.tensor_mul(valid[:, lo:], valid[:, lo:], nv)

            # ---- slot scores: rows . W_align[D:], pipelined per half ----
            prod = big.tile([S, D], f32)
            scpa = small.tile([S, 1], f32)
            scpb = small.tile([S, 1], f32)
            scp = small.tile([S, 1], f32)
            nc.vector.tensor_mul(prod[:, 0:HD], rows0[:], wvec[:, 0:HD])
            nc.vector.reduce_sum(scpa[:], prod[:, 0:HD], axis=mybir.AxisListType.X)
            nc.vector.tensor_mul(prod[:, HD:], rows1[:], wvec[:, HD:])
            nc.vector.reduce_sum(scpb[:], prod[:, HD:], axis=mybir.AxisListType.X)
            nc.vector.tensor_add(scp[:], scpa[:], scpb[:])
            s2 = small.tile([BL, TW], f32)
            nc.sync.dma_start(out=s2[:], in_=scp[:])

            # ---- masked softmax over slots ----
            tmpa = small.tile([BL, TW], f32)
            sin = small.tile([BL, TW], f32)
            nc.vector.tensor_scalar(
                out=tmpa[:], in0=valid[:], scalar1=-1.0, scalar2=-NEG,
                op0=mybir.AluOpType.add, op1=mybir.AluOpType.mult,
            )
            nc.vector.tensor_add(sin[:], s2[:], tmpa[:])
            mx = small.tile([BL, 1], f32)
            nc.vector.reduce_max(mx[:], sin[:], axis=mybir.AxisListType.X)
            bias = small.tile([BL, 1], f32)
            nc.vector.tensor_sub(bias[:], cb[:], mx[:])
            nc.vector.tensor_add(bias[:], bias[:], balb[:])

            ev = small.tile([BL, TW], f32)
            zsum = small.tile([BL, 1], f32)
            nc.scalar.activation(
                out=ev[:], in_=sin[:], func=mybir.ActivationFunctionType.Exp,
                bias=bias[:, 0:1], scale=1.0, accum_out=zsum[:],
            )
            ea = small.tile([BL, TW], f32)
            nc.scalar.activation(
                out=ea[:], in_=s2[:], func=mybir.ActivationFunctionType.Exp,
                bias=bias[:, 0:1], scale=1.0,
            )
            rz = small.tile([BL, 1], f32)
            nc.vector.reciprocal(rz[:], zsum[:])

            pv = small.tile([BL, TW], bf16)
            nc.vector.tensor_scalar(
                out=pv[:], in0=ev[:], scalar1=rz[:, 0:1], scalar2=None,
                op0=mybir.AluOpType.mult,
            )
            wsc = small.tile([BL, TW], f32)
            nc.vector.tensor_scalar(
                out=wsc[:], in0=ea[:], scalar1=rz[:, 0:1], scalar2=None,
                op0=mybir.AluOpType.mult,
            )
            nc.vector.tensor_mul(wsc[:], wsc[:], un[:])

            # ---- pooled[b, :] = probs_b @ rows_b  (probs as tiny lhsT) ----
            # matmul runs in bf16 (fp32 PE streaming is ~8x slower); the
            # rows cast happens on the scalar engine while DVE does scores
            rows_bf = big.tile([S, D], bf16)
            nc.scalar.copy(rows_bf[:, 0:HD], rows0[:])
            nc.scalar.copy(rows_bf[:, HD:], rows1[:])
            probs2 = small.tile([S, BL], bf16)
            nc.gpsimd.memset(probs2[:], 0.0)
            for b in range(BL):
                nc.sync.dma_start(
                    out=probs2[b * TW : (b + 1) * TW, b : b + 1],
                    in_=pv[b : b + 1, :],
                )
            # pooled in d-major layout: rows as stationary per-chunk weights
            pl = ps.tile([128, DC * BL], f32)
            for c in range(DC):
                nc.tensor.matmul(
                    pl[:, c * BL : (c + 1) * BL],
                    lhsT=rows_bf[:, c * 128 : (c + 1) * 128],
                    rhs=probs2[:],
                    start=True,
                    stop=True,
                )
            plsb = small.tile([128, DC * BL], bf16)
            nc.vector.tensor_copy(plsb[:], pl[:])
            nc.sync.dma_start(out=pooledT[:], in_=plsb[:])

            # ---- scatter attention probabilities ----
            wp = small.tile([S, 1], f32)
            nc.sync.dma_start(out=wp[:], in_=wsc[:])
            nc.gpsimd.indirect_dma_start(
                out=attn[:],
                out_offset=bass.IndirectOffsetOnAxis(ap=idxp[:, 0:1], axis=0),
                in_=wp[:],
                in_offset=None,
            )

            if debug:
                for nm, src in [
                    ("d_idxp", idxp[:]), ("d_s2", s2[:]), ("d_un", un[:]),
                    ("d_valid", valid[:]), ("d_pv", pv[:]), ("d_wsc", wsc[:]),
                    ("d_cb", cb[:]), ("d_mx", mx[:]), ("d_zs", zsum[:]),
                    ("d_probs2", probs2[:]), ("d_rows", rows0[:, 0:8]),
                ]:
                    nc.sync.dma_start(out=dbg[nm][:], in_=src)

    return _legalize_multi_waits(nc)


def _build_proj_kernel():
    """Per-core slice of the output projection.

    Inputs:  pooledb [128, DC*B]   (pooledb[p, c*B+b] = pooled[b, c*128+p])
             wob [128, DC*OSH]     (wob[p, c*OSH+o] = W_out[o_base+o, c*128+p])
             bob [1, OSH]
    Output:  projT [B, OSH]        (projT[b, o] = projected[b, o_base+o])
    """
    nc = bass.Bass()
    pooledb = nc.dram_tensor("pooledb", [128, DC * B], bf16, kind="ExternalInput")
    wob = nc.dram_tensor("wob", [128, DC * OSH], bf16, kind="ExternalInput")
    bob = nc.dram_tensor("bob", [1, OSH], f32, kind="ExternalInput")
    projT = nc.dram_tensor("projT", [B, OSH], f32, kind="ExternalOutput")

    NQ = 8
    CQ = DC // NQ
    with tile.TileContext(nc) as tc:
        with (
            tc.tile_pool(name="pool", bufs=1) as pool,
            tc.tile_pool(name="ps", bufs=1, space="PSUM") as ps,
        ):
            pb = pool.tile([128, DC, B], bf16)
            nc.sync.dma_start(
                out=pb[:], in_=pooledb[:].rearrange("p (c b) -> p c b", b=B)
            )
            wq = []
            for q in range(NQ):
                t = pool.tile([128, CQ, OSH], bf16, tag=f"w{q}")
                eng = nc.sync if q % 2 == 0 else nc.scalar
                eng.dma_start(
                    out=t[:],
                    in_=wob[:, q * CQ * OSH : (q + 1) * CQ * OSH].rearrange(
                        "p (c o) -> p c o", o=OSH
                    ),
                )
                wq.append(t)
            # bias folded into the PSUM group as a K=1 ones-matmul
            ones1 = pool.tile([1, B], bf16)
            nc.vector.memset(ones1[:], 1.0)
            bobs = pool.tile([1, OSH], bf16)
            nc.gpsimd.dma_start(out=bobs[:], in_=bob[0:1, :])

            pj = ps.tile([B, OSH], f32)
            nc.tensor.matmul(pj[:], lhsT=ones1[:], rhs=bobs[:], start=True, stop=False)
            for c in range(DC):
                q, r = divmod(c, CQ)
                nc.tensor.matmul(
                    pj[:],
                    lhsT=pb[:, c, :],
                    rhs=wq[q][:, r, :],
                    start=False,
                    stop=(c == DC - 1),
                )
            ot2 = pool.tile([B, OSH], f32)
            nc.scalar.activation(
                out=ot2[:], in_=pj[:], func=mybir.ActivationFunctionType.Tanh
            )
            nc.sync.dma_start(out=projT[:], in_=ot2[:])

    return _legalize_multi_waits(nc)


_POOL_NC = None
_PROJ_NC = None


def _get_kernels():
    global _POOL_NC, _PROJ_NC
    if _POOL_NC is None:
        _POOL_NC = _build_pool_kernel()
        _PROJ_NC = _build_proj_kernel()
    return _POOL_NC, _PROJ_NC


def _jc_const():
    p = np.arange(S, dtype=np.int32)
    return ((p % W) + (p // TW) * L).reshape(S, 1)


def _pool_in_maps(hidden, token_idxs, pooled_entities, W_align, b_align):
    wal = np.ascontiguousarray(W_align, dtype=np.float32).reshape(1, 2 * D)
    bal = np.ascontiguousarray(b_align, dtype=np.float32).reshape(1, 1)
    tok = np.ascontiguousarray(token_idxs.reshape(B, T, 2), dtype=np.int32)
    jc = _jc_const()
    maps = []
    for g in range(NCORES):
        b0 = g * BL
        maps.append(
            {
                "hid": np.ascontiguousarray(
                    hidden[b0 : b0 + BL].reshape(BL * L, D), dtype=np.float32
                ),
                "tok": tok[b0 : b0 + BL],
                "jc": jc,
                "ent": np.ascontiguousarray(
                    pooled_entities[b0 : b0 + BL], dtype=np.float32
                ),
                "wal": wal,
                "bal": bal,
            }
        )
    return maps


def _proj_in_maps(pooled_all, W_out, b_out):
    import ml_dtypes

    # pooledb[p, c*B+b] = pooled_all[b, c*128+p]
    pooledb = np.ascontiguousarray(
        pooled_all.T.reshape(DC, 128, B).transpose(1, 0, 2).reshape(128, DC * B)
    ).astype(ml_dtypes.bfloat16)
    maps = []
    for g in range(NCORES):
        o0 = g * OSH
        wob = np.ascontiguousarray(
            W_out[o0 : o0 + OSH]
            .T.reshape(DC, 128, OSH)
            .transpose(1, 0, 2)
            .reshape(128, DC * OSH)
        ).astype(ml_dtypes.bfloat16)
        maps.append(
            {
                "pooledb": pooledb,
                "wob": wob,
                "bob": np.ascontiguousarray(
                    b_out[o0 : o0 + OSH], dtype=np.float32
                ).reshape(1, OSH),
            }
        )
    return maps


LAST_EXEC_NS = []


def kernel(hidden, token_idxs, pooled_entities, W_align, b_align, W_out, b_out):
    from concourse.bass_utils import run_bass_kernel_spmd

    LAST_EXEC_NS.clear()
    hidden = np.asarray(hidden, dtype=np.float32)
    token_idxs = np.asarray(token_idxs)
    pooled_entities = np.asarray(pooled_entities, dtype=np.float32)
    W_align = np.asarray(W_align, dtype=np.float32)
    b_align = np.asarray(b_align, dtype=np.float32)
    W_out = np.asarray(W_out, dtype=np.float32)
    b_out = np.asarray(b_out, dtype=np.float32)

    if int(np.max(token_idxs[..., 1] - token_idxs[..., 0])) > W:
        # Out-of-contract input (setup_inputs caps span length at 16);
        # fall back to a plain numpy evaluation rather than return garbage.
        return _numpy_reference(
            hidden, token_idxs, pooled_entities, W_align, b_align, W_out, b_out
        )

    pool_nc, proj_nc = _get_kernels()
    core_ids = list(range(NCORES))

    kr_a = run_bass_kernel_spmd(
        pool_nc,
        _pool_in_maps(hidden, token_idxs, pooled_entities, W_align, b_align),
        core_ids,
    )
    res_a = kr_a.results
    LAST_EXEC_NS.append(kr_a.exec_time_ns)

    pooled_all = np.empty((B, D), dtype=np.float32)
    for g in range(NCORES):
        pt = np.asarray(res_a[g]["pooledT"], dtype=np.float32).reshape(128, DC, BL)
        for b in range(BL):
            pooled_all[g * BL + b] = pt[:, :, b].T.reshape(D)
    attn = np.empty((F, B, L, 1), dtype=np.float32)
    for g in range(NCORES):
        attn[0, g * BL : (g + 1) * BL, :, 0] = res_a[g]["attn"].reshape(BL, L)

    kr_b = run_bass_kernel_spmd(
        proj_nc, _proj_in_maps(pooled_all, W_out, b_out), core_ids
    )
    res_b = kr_b.results
    LAST_EXEC_NS.append(kr_b.exec_time_ns)

    projected = np.empty((B, D), dtype=np.float32)
    for g in range(NCORES):
        projected[:, g * OSH : (g + 1) * OSH] = res_b[g]["projT"]
    return projected, attn


def _numpy_reference(hidden, token_idxs, pooled_entities, W_align, b_align, W_out, b_out):
    pos = np.arange(L)
    starts = token_idxs[..., 0][..., None]
    ends = token_idxs[..., 1][..., None]
    mask = ((pos >= starts) & (pos < ends)).any(axis=2)  # [F,B,L]
    attn = np.zeros((F, B, L, 1), dtype=np.float32)
    pooled_f = []
    for f in range(F):
        m = mask[f][..., None].astype(np.float32)
        mh = hidden * m
        er = pooled_entities[:, None, :] * m
        scores = np.concatenate([er, mh], axis=-1) @ W_align.T + b_align
        s = np.where(mask[f], scores[..., 0], -np.inf)
        s = s - s.max(axis=-1, keepdims=True)
        e = np.exp(s)
        probs = e / e.sum(axis=-1, keepdims=True)
        probs = np.where(mask[f], probs, 0.0)
        attn[f, :, :, 0] = probs
        pooled_f.append((mh * probs[..., None]).sum(axis=1))
    all_pooled = np.concatenate(pooled_f, axis=1)
    projected = np.tanh(all_pooled @ W_out.T + b_out).astype(np.float32)
    return projected, attn
